# revision 10
# baseline (speedup 1.0000x reference)
"""AdaptiveGraphConv (Chebyshev K=3) Trainium2 kernel, 8-core data-parallel.

Math (per (batch,time) item, x_item [N,C]):
  M = D^-1/2 A D^-1/2  (normalized adjacency; L = I - M), M symmetric.
  T0 = x; T1 = Lx; T2 = 2L T1 - T0
  out = T0 W0 + T1 W1 + T2 W2 + b
      = x (W0+W1+W2) + (Mx)(-W1-4W2) + (M^2 x)(2W2) + b
M, M^2, the combined weights and the bias are tiny and cheap, so they are
precomputed on the host (numpy) and shipped ready-to-use; the device does
only the heavy work:
  - transpose x to node-major (PE transposes),
  - MX_cm[(b,c), i] = sum_j X_nm[j, (b,c)] * M[j, i]  (X_nm stationary),
    which writes MX / M^2X directly in channel-major, no back-transposes;
    both are stored t-major ([128, T, N], contiguous PSUM evictions),
  - the W stage streaming (t outer, n inner) so every operand is either
    contiguous or a strided *read* (strided writes are the slow case).
Sharding: data-parallel over batch dim B=64 -> 8 batches/core. Graph
matrices, weights, bias replicated. No collectives.
"""
import os
import sys
import numpy as np

_TRN_REPO = "/opt/trn_rl_repo"
if _TRN_REPO not in sys.path:
    sys.path.insert(0, _TRN_REPO)


def _ensure_ntff_hook():
    """Make antenv.axon_hooks importable so NTFF profiling can register.

    The agent container's antenv stub lacks axon_hooks; trn_boot degrades
    silently without it. Writing the tiny registry module before concourse
    imports restores profiling. Harmless if already present.
    """
    src = (
        "_hook = None\n"
        "def set_axon_ntff_profile_hook(hook):\n"
        "    global _hook\n"
        "    _hook = hook\n"
        "def get_axon_ntff_profile_hook():\n"
        "    return _hook\n"
    )
    try:
        import antenv  # noqa
        base = os.path.dirname(antenv.__file__)
        path = os.path.join(base, "axon_hooks.py")
        if not os.path.exists(path):
            with open(path, "w") as f:
                f.write(src)
    except Exception:
        pass


_ensure_ntff_hook()

B, C, N, T, K = 64, 64, 325, 12, 3
NCORES = 8
B_LOC = B // NCORES          # 8 batches per core
NPAIRS = B_LOC // 2          # 4 pairs of batches
NT = N * T                   # 3900
CNT = [128, 128, 69]         # node chunk sizes (325 = 128+128+69)
NOFF = [0, 128, 256]
NBLK = 42                    # W-stage node-block (504 cols <= one PSUM bank)

_cache = {}


def _build():
    import concourse.bass as bass  # noqa
    import concourse.bacc as bacc
    import concourse.mybir as mybir
    import concourse.tile as tile
    from concourse import masks
    from contextlib import ExitStack

    f32 = mybir.dt.float32
    bf16 = mybir.dt.bfloat16
    AF = mybir.ActivationFunctionType

    nc = bacc.Bacc("TRN2", target_bir_lowering=False, debug=False,
                   num_devices=NCORES)
    x_ext = nc.dram_tensor("x", [B_LOC, C, N, T], f32, kind="ExternalInput")
    m_ext = nc.dram_tensor("m", [3, 2, 128, N], bf16, kind="ExternalInput")
    w_ext = nc.dram_tensor("w", [K, 128, 128], bf16, kind="ExternalInput")
    b_ext = nc.dram_tensor("bias", [128, 1], f32, kind="ExternalInput")
    out_ext = nc.dram_tensor("out", [B_LOC, C, N, T], f32,
                             kind="ExternalOutput")

    with tile.TileContext(nc) as tc, ExitStack() as ctx:
        sb = ctx.enter_context(tc.tile_pool(name="sb", bufs=2))
        psp = ctx.enter_context(tc.tile_pool(name="psp", bufs=2,
                                             space="PSUM"))

        state = {}

        def emit_loads(p):
            Xf = sb.tile([128, N, T], f32, tag="xf", name="xf")
            if p == 0:
                for i in range(3):
                    nsl = slice(NOFF[i], NOFF[i] + CNT[i])
                    for h in (0, 1):
                        nc.sync.dma_start(Xf[64 * h: 64 * h + 64, nsl, :],
                                          x_ext.ap()[2 * p + h, :, nsl, :])
            else:
                for h in (0, 1):
                    nc.sync.dma_start(
                        Xf[64 * h: 64 * h + 64, :, :].rearrange(
                            "p n t -> p (n t)"),
                        x_ext.ap()[2 * p + h].rearrange("c n t -> c (n t)"))
            state[p] = Xf

        # graph matrices / weights / bias, host-precomputed, one DMA each
        Mt = sb.tile([128, 3, 2, N], bf16, bufs=1)
        nc.sync.dma_start(Mt[:], m_ext.ap().rearrange("j k p n -> p j k n"))
        Wt = sb.tile([128, 3, 128], bf16, bufs=1)
        nc.sync.dma_start(Wt[:], w_ext.ap().rearrange("k p n -> p k n"))
        bias = sb.tile([128, 1], f32, bufs=1)
        nc.sync.dma_start(bias[:], b_ext.ap())

        emit_loads(0)

        M = [Mt[: CNT[j], j, 0, :] for j in range(3)]
        M2 = [Mt[: CNT[j], j, 1, :] for j in range(3)]
        Wa, Wb, Wc = (Wt[:, k, :] for k in range(3))

        idn = sb.tile([128, 128], bf16, bufs=1)
        masks.make_identity(nc, idn[:])

        def emit_convert_piece(p, i):
            # f32 (n,t) -> bf16 (t,n): the reorder rides on the strided READ
            # (strided reads are cheap; strided writes are not).
            if (p, "xs") not in state:
                state[(p, "xs")] = sb.tile([128, T, N], bf16, tag="xsb",
                                           name="xsb")
                state[(p, "nconv")] = 0
            Xf = state[p]
            Xs = state[(p, "xs")]
            nsl = slice(NOFF[i], NOFF[i] + CNT[i])
            srcv = Xf[:, nsl, :].rearrange("p n t -> p t n")
            if i == 0:
                nc.scalar.activation(Xs[:, :, nsl], srcv, AF.Copy)
            elif i == 1:
                nc.vector.tensor_copy(Xs[:, :, nsl], srcv)
            else:
                nc.gpsimd.tensor_copy(Xs[:, :, nsl], srcv)
            state[(p, "nconv")] += 1
            if state[(p, "nconv")] == 3:
                state.pop(p)
                state.pop((p, "nconv"))
                state[p] = state.pop((p, "xs"))

        def emit_convert(p):
            for i in range(3):
                emit_convert_piece(p, i)

        emit_convert(0)
        for p in range(NPAIRS):
            Xs = state.pop(p)
            if p + 1 < NPAIRS:
                emit_loads(p + 1)
                emit_convert_piece(p + 1, 2)

            # node-major X: 3 tiles [n<=128, T, 128=(2b,c)]
            XN = [sb.tile([128, T, 128], bf16, tag=f"xn{i}",
                          name=f"xn{i}") for i in range(3)]
            for i in range(3):
                nsl = slice(NOFF[i], NOFF[i] + CNT[i])
                for tg in range(2):
                    ps = psp.tile([128, 6, 128], bf16, tag="pst", bufs=2)
                    for tt in range(6):
                        t = tg * 6 + tt
                        nc.tensor.matmul(
                            ps[: CNT[i], tt, :],
                            Xs[:, t, nsl],
                            idn[:], is_transpose=True)
                    if tg == 0:
                        nc.scalar.activation(
                            XN[i][: CNT[i], 0:6, :],
                            ps[: CNT[i], :, :], AF.Copy)
                    else:
                        nc.vector.tensor_copy(
                            XN[i][: CNT[i], 6:12, :],
                            ps[: CNT[i], :, :])

            # M-apply: MX and M2X in channel-major, t-major storage.
            # psA[(2b,c), i] = sum_j XN[j][t,(2b,c)] * M[j][:, i]
            MX = sb.tile([128, T, N], bf16, tag="mx")
            M2X = sb.tile([128, T, N], bf16, tag="m2x")
            for t in range(T):
                psA = psp.tile([128, N], f32, tag="psm", bufs=4)
                psB = psp.tile([128, N], f32, tag="psm", bufs=4)
                for j in range(3):
                    lhsT = XN[j][: CNT[j], t, :]
                    nc.tensor.matmul(psA[:, :], lhsT, M[j],
                                     start=(j == 0), stop=(j == 2))
                    nc.tensor.matmul(psB[:, :], lhsT, M2[j],
                                     start=(j == 0), stop=(j == 2))
                if t % 2 == 0:
                    nc.vector.tensor_copy(MX[:, t, :], psA[:, :])
                    nc.scalar.activation(M2X[:, t, :], psB[:, :], AF.Copy)
                else:
                    nc.scalar.activation(MX[:, t, :], psA[:, :], AF.Copy)
                    nc.vector.tensor_copy(M2X[:, t, :], psB[:, :])
                if p + 1 < NPAIRS and t in (5, 7):
                    emit_convert_piece(p + 1, {5: 0, 7: 1}[t])

            # W stage: out = Xs*Wa + MX*Wb + M2X*Wc + bias, in n-blocks.
            # Moving operands stream (t outer, n inner): all three rhs are
            # runs-of-nb contiguous reads; the psum holds (t, n) order and
            # the eviction does a strided PSUM read + contiguous SBUF write,
            # split across both engines.
            HALF = 4 * NBLK * T                      # 2016 cols (blocks 0-3)
            outA = sb.tile([128, HALF], f32, tag="outA")
            outB = sb.tile([128, NT - HALF], f32, tag="outB")
            for blk in range(8):
                nb0 = blk * NBLK
                nb = min(NBLK, N - nb0)
                ps = psp.tile([128, T, nb], f32, tag="psw", bufs=2)
                pw = ps[:, :, :]
                ra = Xs[:, :, nb0: nb0 + nb]
                rb = MX[:, :, nb0: nb0 + nb]
                rc = M2X[:, :, nb0: nb0 + nb]
                nc.tensor.matmul(pw, Wa, ra, start=True, stop=False)
                nc.tensor.matmul(pw, Wb, rb, start=False, stop=False)
                nc.tensor.matmul(pw, Wc, rc, start=False, stop=True)
                if blk < 4:
                    dst = outA[:, blk * NBLK * T: (blk + 1) * NBLK * T]
                else:
                    dst = outB[:, (blk - 4) * NBLK * T:
                               (blk - 4) * NBLK * T + nb * T]
                nh = nb // 2
                pr0 = ps[:, :, :nh].rearrange("p t n -> p n t")
                pr1 = ps[:, :, nh:].rearrange("p t n -> p n t")
                nc.scalar.activation(dst[:, : nh * T], pr0, AF.Identity,
                                     bias=bias[:, 0:1])
                nc.vector.tensor_scalar_add(dst[:, nh * T:], pr1,
                                            bias[:, 0:1])

            out_hbm = [out_ext.ap()[2 * p + h].rearrange("c n t -> c (n t)")
                       for h in (0, 1)]
            for h in (0, 1):
                nc.sync.dma_start(out_hbm[h][:, :HALF],
                                  outA[64 * h: 64 * h + 64, :])
            for h in (0, 1):
                eng = nc.gpsimd if p < NPAIRS - 1 else nc.sync
                eng.dma_start(out_hbm[h][:, HALF:],
                              outB[64 * h: 64 * h + 64, :])

    nc.compile()
    return nc


def _get_nc():
    if "nc" not in _cache:
        _cache["nc"] = _build()
    return _cache["nc"]


last_exec_time_ns = None
last_results = None


def kernel(x, adj, W, b):
    import ml_dtypes
    from concourse.bass_utils import run_bass_kernel_spmd

    global last_exec_time_ns, last_results
    nc = _get_nc()
    x = np.ascontiguousarray(x, dtype=np.float32)
    adj = np.asarray(adj, dtype=np.float64)
    W = np.asarray(W, dtype=np.float64)
    b = np.asarray(b, dtype=np.float64)

    # host-side precompute of the tiny graph/weight tensors
    d = adj.sum(axis=-1)
    s = np.where(d > 0, 1.0 / np.sqrt(d), 0.0)
    M = s[:, None] * adj * s[None, :]
    M2 = M @ M
    bf16 = ml_dtypes.bfloat16
    m_in = np.zeros((3, 2, 128, N), dtype=bf16)
    for j in range(3):
        m_in[j, 0, : CNT[j]] = M[NOFF[j]: NOFF[j] + CNT[j], :].astype(bf16)
        m_in[j, 1, : CNT[j]] = M2[NOFF[j]: NOFF[j] + CNT[j], :].astype(bf16)

    Wcomb = (W[0] + W[1] + W[2], -W[1] - 4.0 * W[2], 2.0 * W[2])
    w_in = np.zeros((K, 128, 128), dtype=bf16)
    for k, Wx in enumerate(Wcomb):
        w_in[k, :64, :64] = Wx.astype(bf16)
        w_in[k, 64:, 64:] = Wx.astype(bf16)
    b_in = np.ascontiguousarray(
        np.tile(b.astype(np.float32), 2).reshape(128, 1))

    in_maps = [
        {"x": x[i * B_LOC: (i + 1) * B_LOC],
         "m": m_in, "w": w_in, "bias": b_in}
        for i in range(NCORES)
    ]
    trace = bool(os.environ.get("KERNEL_TRACE"))
    res = run_bass_kernel_spmd(nc, in_maps, list(range(NCORES)), trace=trace)
    last_exec_time_ns = res.exec_time_ns
    last_results = res
    out = np.concatenate([res.results[i]["out"] for i in range(NCORES)],
                         axis=0)
    return out


# revision 11
# speedup vs baseline: 1.1810x; 1.1810x over previous
"""AdaptiveGraphConv (Chebyshev K=3) Trainium2 kernel, 8-core data-parallel.

Math (per (batch,time) item, x_item [N,C]):
  M = D^-1/2 A D^-1/2  (normalized adjacency; L = I - M), M symmetric.
  T0 = x; T1 = Lx; T2 = 2L T1 - T0
  out = T0 W0 + T1 W1 + T2 W2 + b
      = x (W0+W1+W2) + (Mx)(-W1-4W2) + (M^2 x)(2W2) + b
M, M^2, the combined weights and the bias are tiny and cheap, so they are
precomputed on the host (numpy) and shipped ready-to-use; the device does
only the heavy work:
  - transpose x to node-major (PE transposes),
  - MX_cm[(b,c), i] = sum_j X_nm[j, (b,c)] * M[j, i]  (X_nm stationary),
    which writes MX / M^2X directly in channel-major, no back-transposes;
    both are stored t-major ([128, T, N], contiguous PSUM evictions),
  - the W stage streaming (t outer, n inner) so every operand is either
    contiguous or a strided *read* (strided writes are the slow case).
Sharding: data-parallel over batch dim B=64 -> 8 batches/core. Graph
matrices, weights, bias replicated. No collectives.
"""
import os
import sys
import numpy as np

_TRN_REPO = "/opt/trn_rl_repo"
if _TRN_REPO not in sys.path:
    sys.path.insert(0, _TRN_REPO)


def _ensure_ntff_hook():
    """Make antenv.axon_hooks importable so NTFF profiling can register.

    The agent container's antenv stub lacks axon_hooks; trn_boot degrades
    silently without it. Writing the tiny registry module before concourse
    imports restores profiling. Harmless if already present.
    """
    src = (
        "_hook = None\n"
        "def set_axon_ntff_profile_hook(hook):\n"
        "    global _hook\n"
        "    _hook = hook\n"
        "def get_axon_ntff_profile_hook():\n"
        "    return _hook\n"
    )
    try:
        import antenv  # noqa
        base = os.path.dirname(antenv.__file__)
        path = os.path.join(base, "axon_hooks.py")
        if not os.path.exists(path):
            with open(path, "w") as f:
                f.write(src)
    except Exception:
        pass


_ensure_ntff_hook()

B, C, N, T, K = 64, 64, 325, 12, 3
NCORES = 8
B_LOC = B // NCORES          # 8 batches per core
NPAIRS = B_LOC // 2          # 4 pairs of batches
NT = N * T                   # 3900
CNT = [128, 128, 69]         # node chunk sizes (325 = 128+128+69)
NOFF = [0, 128, 256]
NBLK = 42                    # W-stage node-block (504 cols <= one PSUM bank)

_cache = {}


def _build():
    import concourse.bass as bass  # noqa
    import concourse.bacc as bacc
    import concourse.mybir as mybir
    import concourse.tile as tile
    from concourse import masks
    from contextlib import ExitStack

    f32 = mybir.dt.float32
    bf16 = mybir.dt.bfloat16
    AF = mybir.ActivationFunctionType

    nc = bacc.Bacc("TRN2", target_bir_lowering=False, debug=False,
                   num_devices=NCORES)
    x_ext = nc.dram_tensor("x", [B_LOC, C, N, T], f32, kind="ExternalInput")
    m_ext = nc.dram_tensor("m", [3, 2, 128, N], bf16, kind="ExternalInput")
    w_ext = nc.dram_tensor("w", [K, 128, 128], bf16, kind="ExternalInput")
    b_ext = nc.dram_tensor("bias", [128, 1], f32, kind="ExternalInput")
    out_ext = nc.dram_tensor("out", [B_LOC, C, N, T], f32,
                             kind="ExternalOutput")

    with tile.TileContext(nc) as tc, ExitStack() as ctx:
        const = ctx.enter_context(tc.tile_pool(name="const", bufs=1))
        ps_t = ctx.enter_context(
            tc.tile_pool(name="ps_t", bufs=2, space="PSUM"))
        ps_m = ctx.enter_context(
            tc.tile_pool(name="ps_m", bufs=4, space="PSUM"))
        ps_w = ctx.enter_context(
            tc.tile_pool(name="ps_w", bufs=2, space="PSUM"))
        xs_pool = ctx.enter_context(tc.tile_pool(name="xs", bufs=2))
        nm_pool = ctx.enter_context(tc.tile_pool(name="nm", bufs=2))
        cm_pool = ctx.enter_context(tc.tile_pool(name="cm", bufs=2))
        out_pool = ctx.enter_context(tc.tile_pool(name="outp", bufs=2))

        state = {}

        def emit_loads(p):
            Xf = xs_pool.tile([128, N, T], f32, tag="xf", name="xf")
            if p == 0:
                for i in range(3):
                    nsl = slice(NOFF[i], NOFF[i] + CNT[i])
                    for h in (0, 1):
                        nc.sync.dma_start(Xf[64 * h: 64 * h + 64, nsl, :],
                                          x_ext.ap()[2 * p + h, :, nsl, :])
            else:
                for h in (0, 1):
                    nc.sync.dma_start(
                        Xf[64 * h: 64 * h + 64, :, :].rearrange(
                            "p n t -> p (n t)"),
                        x_ext.ap()[2 * p + h].rearrange("c n t -> c (n t)"))
            state[p] = Xf

        # graph matrices / weights / bias, host-precomputed, one DMA each
        Mt = const.tile([128, 3, 2, N], bf16)
        nc.sync.dma_start(Mt[:], m_ext.ap().rearrange("j k p n -> p j k n"))
        Wt = const.tile([128, 3, 128], bf16)
        nc.sync.dma_start(Wt[:], w_ext.ap().rearrange("k p n -> p k n"))
        bias = const.tile([128, 1], f32)
        nc.sync.dma_start(bias[:], b_ext.ap())

        emit_loads(0)

        M = [Mt[: CNT[j], j, 0, :] for j in range(3)]
        M2 = [Mt[: CNT[j], j, 1, :] for j in range(3)]
        Wa, Wb, Wc = (Wt[:, k, :] for k in range(3))

        idn = const.tile([128, 128], bf16)
        masks.make_identity(nc, idn[:])

        def emit_convert_piece(p, i):
            # f32 (n,t) -> bf16 (t,n): the reorder rides on the strided READ
            # (strided reads are cheap; strided writes are not).
            if (p, "xs") not in state:
                state[(p, "xs")] = xs_pool.tile([128, T, N], bf16, tag="xsb",
                                                name="xsb")
                state[(p, "nconv")] = 0
            Xf = state[p]
            Xs = state[(p, "xs")]
            nsl = slice(NOFF[i], NOFF[i] + CNT[i])
            srcv = Xf[:, nsl, :].rearrange("p n t -> p t n")
            if i == 0:
                nc.scalar.activation(Xs[:, :, nsl], srcv, AF.Copy)
            elif i == 1:
                nc.vector.tensor_copy(Xs[:, :, nsl], srcv)
            else:
                nc.gpsimd.tensor_copy(Xs[:, :, nsl], srcv)
            state[(p, "nconv")] += 1
            if state[(p, "nconv")] == 3:
                state.pop(p)
                state.pop((p, "nconv"))
                state[p] = state.pop((p, "xs"))

        def emit_convert(p):
            for i in range(3):
                emit_convert_piece(p, i)

        emit_convert(0)
        for p in range(NPAIRS):
            Xs = state.pop(p)
            if p + 1 < NPAIRS:
                emit_loads(p + 1)
                emit_convert_piece(p + 1, 2)

            # node-major X: 3 tiles [n<=128, T, 128=(2b,c)]
            XN = [nm_pool.tile([128, T, 128], bf16, tag=f"xn{i}",
                               name=f"xn{i}") for i in range(3)]
            for i in range(3):
                nsl = slice(NOFF[i], NOFF[i] + CNT[i])
                for tg in range(2):
                    ps = ps_t.tile([128, 6, 128], bf16, tag="pst")
                    for tt in range(6):
                        t = tg * 6 + tt
                        nc.tensor.matmul(
                            ps[: CNT[i], tt, :],
                            Xs[:, t, nsl],
                            idn[:], is_transpose=True)
                    if tg == 0:
                        nc.scalar.activation(
                            XN[i][: CNT[i], 0:6, :],
                            ps[: CNT[i], :, :], AF.Copy)
                    else:
                        nc.vector.tensor_copy(
                            XN[i][: CNT[i], 6:12, :],
                            ps[: CNT[i], :, :])

            # M-apply: MX and M2X in channel-major, t-major storage.
            # psA[(2b,c), i] = sum_j XN[j][t,(2b,c)] * M[j][:, i]
            MX = cm_pool.tile([128, T, N], bf16, tag="mx")
            M2X = cm_pool.tile([128, T, N], bf16, tag="m2x")
            for t in range(T):
                psA = ps_m.tile([128, N], f32, tag="psm")
                psB = ps_m.tile([128, N], f32, tag="psm")
                for j in range(3):
                    lhsT = XN[j][: CNT[j], t, :]
                    nc.tensor.matmul(psA[:, :], lhsT, M[j],
                                     start=(j == 0), stop=(j == 2))
                    nc.tensor.matmul(psB[:, :], lhsT, M2[j],
                                     start=(j == 0), stop=(j == 2))
                if t % 2 == 0:
                    nc.vector.tensor_copy(MX[:, t, :], psA[:, :])
                    nc.scalar.activation(M2X[:, t, :], psB[:, :], AF.Copy)
                else:
                    nc.scalar.activation(MX[:, t, :], psA[:, :], AF.Copy)
                    nc.vector.tensor_copy(M2X[:, t, :], psB[:, :])
                if p + 1 < NPAIRS and t in (5, 7):
                    emit_convert_piece(p + 1, {5: 0, 7: 1}[t])

            # W stage: out = Xs*Wa + MX*Wb + M2X*Wc + bias, in n-blocks.
            # Moving operands stream (t outer, n inner): all three rhs are
            # runs-of-nb contiguous reads; the psum holds (t, n) order and
            # the eviction does a strided PSUM read + contiguous SBUF write,
            # split across both engines.
            HALF = 4 * NBLK * T                      # 2016 cols (blocks 0-3)
            outA = out_pool.tile([128, HALF], f32, tag="outA")
            outB = out_pool.tile([128, NT - HALF], f32, tag="outB")
            for blk in range(8):
                nb0 = blk * NBLK
                nb = min(NBLK, N - nb0)
                ps = ps_w.tile([128, T, nb], f32, tag="psw")
                pw = ps[:, :, :]
                ra = Xs[:, :, nb0: nb0 + nb]
                rb = MX[:, :, nb0: nb0 + nb]
                rc = M2X[:, :, nb0: nb0 + nb]
                nc.tensor.matmul(pw, Wa, ra, start=True, stop=False)
                nc.tensor.matmul(pw, Wb, rb, start=False, stop=False)
                nc.tensor.matmul(pw, Wc, rc, start=False, stop=True)
                if blk < 4:
                    dst = outA[:, blk * NBLK * T: (blk + 1) * NBLK * T]
                else:
                    dst = outB[:, (blk - 4) * NBLK * T:
                               (blk - 4) * NBLK * T + nb * T]
                nh = nb // 2
                pr0 = ps[:, :, :nh].rearrange("p t n -> p n t")
                pr1 = ps[:, :, nh:].rearrange("p t n -> p n t")
                nc.scalar.activation(dst[:, : nh * T], pr0, AF.Identity,
                                     bias=bias[:, 0:1])
                nc.vector.tensor_scalar_add(dst[:, nh * T:], pr1,
                                            bias[:, 0:1])

            out_hbm = [out_ext.ap()[2 * p + h].rearrange("c n t -> c (n t)")
                       for h in (0, 1)]
            for h in (0, 1):
                nc.sync.dma_start(out_hbm[h][:, :HALF],
                                  outA[64 * h: 64 * h + 64, :])
            for h in (0, 1):
                eng = nc.gpsimd if p < NPAIRS - 1 else nc.sync
                eng.dma_start(out_hbm[h][:, HALF:],
                              outB[64 * h: 64 * h + 64, :])

    nc.compile()
    return nc


def _get_nc():
    if "nc" not in _cache:
        _cache["nc"] = _build()
    return _cache["nc"]


last_exec_time_ns = None
last_results = None


def kernel(x, adj, W, b):
    import ml_dtypes
    from concourse.bass_utils import run_bass_kernel_spmd

    global last_exec_time_ns, last_results
    nc = _get_nc()
    x = np.ascontiguousarray(x, dtype=np.float32)
    adj = np.asarray(adj, dtype=np.float64)
    W = np.asarray(W, dtype=np.float64)
    b = np.asarray(b, dtype=np.float64)

    # host-side precompute of the tiny graph/weight tensors
    d = adj.sum(axis=-1)
    s = np.where(d > 0, 1.0 / np.sqrt(d), 0.0)
    M = s[:, None] * adj * s[None, :]
    M2 = M @ M
    bf16 = ml_dtypes.bfloat16
    m_in = np.zeros((3, 2, 128, N), dtype=bf16)
    for j in range(3):
        m_in[j, 0, : CNT[j]] = M[NOFF[j]: NOFF[j] + CNT[j], :].astype(bf16)
        m_in[j, 1, : CNT[j]] = M2[NOFF[j]: NOFF[j] + CNT[j], :].astype(bf16)

    Wcomb = (W[0] + W[1] + W[2], -W[1] - 4.0 * W[2], 2.0 * W[2])
    w_in = np.zeros((K, 128, 128), dtype=bf16)
    for k, Wx in enumerate(Wcomb):
        w_in[k, :64, :64] = Wx.astype(bf16)
        w_in[k, 64:, 64:] = Wx.astype(bf16)
    b_in = np.ascontiguousarray(
        np.tile(b.astype(np.float32), 2).reshape(128, 1))

    in_maps = [
        {"x": x[i * B_LOC: (i + 1) * B_LOC],
         "m": m_in, "w": w_in, "bias": b_in}
        for i in range(NCORES)
    ]
    trace = bool(os.environ.get("KERNEL_TRACE"))
    res = run_bass_kernel_spmd(nc, in_maps, list(range(NCORES)), trace=trace)
    last_exec_time_ns = res.exec_time_ns
    last_results = res
    out = np.concatenate([res.results[i]["out"] for i in range(NCORES)],
                         axis=0)
    return out
